# revision 11
# baseline (speedup 1.0000x reference)
"""kNN edge-feature kernel (PoseNet-style GNN message passing) for Trainium2.

Problem: given cloud [8, 3, 4096] f32, for each batch element compute the
K=16 nearest neighbors of every point (by squared euclidean distance, self
included) and emit edge features [8, 6, 4096, 16]:
  out[b, 0:3, n, k] = cloud[b, :, n]                      (central, broadcast)
  out[b, 3:6, n, k] = cloud[b, :, idx[n,k]] - cloud[b, :, n]

Sharding: data-parallel over batch; core b handles batch element b.

Per-core device algorithm, per 128-row tile:
  - negdist[n, m] = 2 x_n.x_m - |x_n|^2 - |x_m|^2  (= -squared distance)
    on the PE as a 5-deep contraction of host-augmented matrices.
  - top-16 per row on the DVE via max8 / max_index8 / match_replace8
    (two rounds of 8; rank 0 is the self-match).
  - neighbor coords via gpsimd ap_gather from a broadcast [128, 3*4096]
    copy of the cloud; the per-16-partition wrapped index semantics are
    resolved with a constant mask + strided sum-reduce.
  - edge assembly with small per-tile vector ops; strided DMA store.
"""

import numpy as np

import concourse.bacc as bacc
import concourse.bass as bass
import concourse.mybir as mybir
from concourse.tile import TileContext

B, C, N, K = 8, 3, 4096, 16
P = 128            # rows per tile (SBUF partitions)
NT = N // P        # 32 row tiles
FCH = 512          # matmul moving free-dim chunk
NCH = N // FCH     # 8 chunks
NEG = -3.0e38      # match_replace sentinel
NE = C * N         # ap_gather num_elems (12288)
NI = 768           # ap_gather num_idxs per 16-partition group (16 rows * 48)

F32 = mybir.dt.float32
U16 = mybir.dt.uint16
S16 = mybir.dt.int16

MODE = "ap"        # "ap" = full on-device; "host" = indices only, host gather


def build_program():
    nc = bacc.Bacc(trn_type="TRN2")
    lhs_d = nc.dram_tensor("lhs_aug", [5, N], F32, kind="ExternalInput")
    rhs_d = nc.dram_tensor("rhs_aug", [5, N], F32, kind="ExternalInput")
    bcast_d = nc.dram_tensor("bcast", [P, NE], F32, kind="ExternalInput")
    ctrt_d = nc.dram_tensor("ctrt", [NT, P, 16], F32, kind="ExternalInput")
    gmask_d = nc.dram_tensor("gmask", [P, NI], F32, kind="ExternalInput")
    out_d = nc.dram_tensor("out", [2 * C, N, K], F32, kind="ExternalOutput")
    if MODE == "host":
        oidx_d = nc.dram_tensor("oidx", [NT, P, 48], U16, kind="ExternalOutput")

    with TileContext(nc) as tc:
        with (
            tc.tile_pool(name="persist", bufs=1) as persist,
            tc.tile_pool(name="nd", bufs=2) as ndpool,
            tc.tile_pool(name="mm", bufs=8, space="PSUM") as mmpool,
            tc.tile_pool(name="small", bufs=3) as small,
        ):
            lhs_sb = persist.tile([5, N], F32)
            rhs_sb = persist.tile([5, N], F32)
            nc.sync.dma_start(lhs_sb[:], lhs_d[:])
            nc.sync.dma_start(rhs_sb[:], rhs_d[:])
            bcast = persist.tile([P, NE], F32)
            nc.sync.dma_start(bcast[:], bcast_d[:])
            gmask = persist.tile([P, NI], F32)
            nc.sync.dma_start(gmask[:], gmask_d[:])

            for t in range(NT):
                nd = ndpool.tile([P, N], F32, tag="nd")
                for j in range(NCH):
                    ps = mmpool.tile([P, FCH], F32, tag="ps")
                    nc.tensor.matmul(
                        ps[:],
                        lhs_sb[:, t * P:(t + 1) * P],
                        rhs_sb[:, j * FCH:(j + 1) * FCH],
                        start=True,
                        stop=True,
                    )
                    nc.scalar.copy(nd[:, j * FCH:(j + 1) * FCH], ps[:])

                # two rounds of top-8 (descending negdist = ascending distance)
                v1 = small.tile([P, 8], F32, tag="v1")
                v2 = small.tile([P, 8], F32, tag="v2")
                idx = small.tile([P, 48], U16, tag="idx")
                nc.vector.max(out=v1[:], in_=nd[:])
                nc.vector.max_index(out=idx[:, 0:8], in_max=v1[:], in_values=nd[:])
                nc.vector.match_replace(
                    out=nd[:], in_to_replace=v1[:], in_values=nd[:], imm_value=NEG
                )
                nc.vector.max(out=v2[:], in_=nd[:])
                nc.vector.max_index(out=idx[:, 8:16], in_max=v2[:], in_values=nd[:])

                if MODE == "host":
                    nc.sync.dma_start(oidx_d[t], idx[:])
                    continue

                # y/z channel pick positions: +N, +2N
                nc.vector.tensor_scalar_add(idx[:, 16:32], idx[:, 0:16], N)
                nc.vector.tensor_scalar_add(idx[:, 32:48], idx[:, 0:16], 2 * N)

                # gather: each 16-partition group reads its wrapped list;
                # out[p, 16j+q] = bcast[p, idx[16g+q, j]]   (g = p//16)
                g = small.tile([P, NI], F32, tag="g")
                nc.gpsimd.ap_gather(
                    out_ap=g[:],
                    in_ap=bcast[:],
                    idxs_ap=idx[:].bitcast(S16),
                    channels=P,
                    num_elems=NE,
                    d=1,
                    num_idxs=NI,
                )
                # keep only q == p%16 entries, then sum the 16 q-slots away
                nc.vector.tensor_mul(g[:], g[:], gmask[:])
                _g = g[:]
                g3 = bass.AP(_g.tensor, _g.offset, [_g.ap[0], [16, 48], [1, 16]])
                nbr = small.tile([P, 48], F32, tag="nbr")
                nc.vector.tensor_reduce(
                    out=nbr[:], in_=g3, op=mybir.AluOpType.add,
                    axis=mybir.AxisListType.X,
                )

                ctr = small.tile([P, 16], F32, tag="ctr")
                nc.sync.dma_start(ctr[:], ctrt_d[t])

                ot = small.tile([P, 2 * C, K], F32, tag="ot")
                nc.vector.memset(ot[:, 0:C, :], 0.0)
                for c in range(C):
                    nc.vector.tensor_scalar_add(
                        ot[:, c, :], ot[:, c, :], ctr[:, c:c + 1]
                    )
                    nc.vector.tensor_scalar(
                        out=ot[:, C + c, :],
                        in0=nbr[:, c * K:(c + 1) * K],
                        scalar1=ctr[:, c:c + 1],
                        scalar2=None,
                        op0=mybir.AluOpType.subtract,
                    )
                nc.sync.dma_start(
                    out_d[:, t * P:(t + 1) * P, :].rearrange("c n k -> n c k"),
                    ot[:],
                )
    nc.compile()
    return nc


_nc_cache = None


def _get_nc():
    global _nc_cache
    if _nc_cache is None:
        _nc_cache = build_program()
    return _nc_cache


def make_in_maps(cloud: np.ndarray):
    cloud = np.ascontiguousarray(cloud, dtype=np.float32)
    assert cloud.shape == (B, C, N), cloud.shape

    # constant mask: gmask[p, 16j+q] = (q == p%16)
    q = np.arange(NI) % 16
    pm = np.arange(P)[:, None] % 16
    gmask = (q[None, :] == pm).astype(np.float32)

    in_maps = []
    for b in range(B):
        cb = cloud[b]
        sq = np.sum(cb * cb, axis=0, dtype=np.float32)
        lhs = np.empty((5, N), np.float32)
        lhs[0:3] = 2.0 * cb
        lhs[3] = -1.0
        lhs[4] = -sq
        rhs = np.empty((5, N), np.float32)
        rhs[0:3] = cb
        rhs[3] = sq
        rhs[4] = 1.0
        bcast = np.broadcast_to(cb.reshape(1, NE), (P, NE))
        ctrt = np.zeros((NT, P, 16), np.float32)
        ctrt[:, :, 0:C] = cb.T.reshape(NT, P, C)
        in_maps.append(
            {
                "lhs_aug": lhs,
                "rhs_aug": rhs,
                "bcast": np.ascontiguousarray(bcast),
                "ctrt": ctrt,
                "gmask": gmask,
            }
        )
    return in_maps


_runner_cache = None


def _get_runner():
    """Cached jitted 8-core SPMD executor (mirrors bass2jax.run_bass_via_pjrt
    but reusable across calls so repeated runs don't re-trace)."""
    global _runner_cache
    if _runner_cache is not None:
        return _runner_cache

    import jax
    import numpy as _np
    from jax.sharding import Mesh, PartitionSpec
    from jax.experimental.shard_map import shard_map
    from concourse.bass2jax import (
        _bass_exec_p,
        install_neuronx_cc_hook,
        partition_id_tensor,
    )
    import concourse.mybir as _mybir

    nc = _get_nc()
    install_neuronx_cc_hook()
    partition_name = nc.partition_id_tensor.name if nc.partition_id_tensor else None

    in_names, out_names, out_avals, zero_outs = [], [], [], []
    for alloc in nc.m.functions[0].allocations:
        if not isinstance(alloc, _mybir.MemoryLocationSet):
            continue
        name = alloc.memorylocations[0].name
        if alloc.kind == "ExternalInput":
            if name != partition_name:
                in_names.append(name)
        elif alloc.kind == "ExternalOutput":
            shape = tuple(alloc.tensor_shape)
            dtype = _mybir.dt.np(alloc.dtype)
            out_names.append(name)
            out_avals.append(jax.core.ShapedArray(shape, dtype))
            zero_outs.append(_np.zeros(shape, dtype))
    n_params = len(in_names)
    n_outs = len(out_avals)
    all_in_names = list(in_names) + list(out_names)
    if partition_name is not None:
        all_in_names.append(partition_name)

    def _body(*args):
        operands = list(args)
        if partition_name is not None:
            operands.append(partition_id_tensor())
        outs = _bass_exec_p.bind(
            *operands,
            out_avals=tuple(out_avals),
            in_names=tuple(all_in_names),
            out_names=tuple(out_names),
            lowering_input_output_aliases=(),
            sim_require_finite=True,
            sim_require_nnan=True,
            nc=nc,
        )
        return tuple(outs)

    devices = jax.devices()[:B]
    mesh = Mesh(_np.asarray(devices), ("core",))
    in_specs = (PartitionSpec("core"),) * (n_params + n_outs)
    out_specs = (PartitionSpec("core"),) * n_outs
    sharded = jax.jit(
        shard_map(
            _body, mesh=mesh, in_specs=in_specs, out_specs=out_specs, check_rep=False
        ),
        keep_unused=True,
    )

    def runner(in_maps):
        per_core = [[np.asarray(m[name]) for name in in_names] for m in in_maps]
        concat_in = [
            np.concatenate([per_core[c][i] for c in range(B)], axis=0)
            for i in range(n_params)
        ]
        concat_zeros = [
            np.zeros((B * z.shape[0], *z.shape[1:]), z.dtype) for z in zero_outs
        ]
        out_arrs = sharded(*concat_in, *concat_zeros)
        return [
            {
                name: np.asarray(out_arrs[i]).reshape(B, *out_avals[i].shape)[c]
                for i, name in enumerate(out_names)
            }
            for c in range(B)
        ]

    _runner_cache = runner
    return runner


def run(cloud: np.ndarray):
    """Returns out [8, 6, 4096, 16] f32."""
    cloud = np.ascontiguousarray(cloud, dtype=np.float32)
    in_maps = make_in_maps(cloud)
    results = _get_runner()(in_maps)
    if MODE == "host":
        out = np.empty((B, 2 * C, N, K), np.float32)
        for b in range(B):
            idx = results[b]["oidx"].reshape(N, 48)[:, 0:K].astype(np.int64)
            cb = cloud[b]                       # [3, N]
            nbr = cb[:, idx]                    # [3, N, K]
            ctr = cb[:, :, None]
            out[b, 0:C] = np.broadcast_to(ctr, (C, N, K))
            out[b, C:] = nbr - ctr
        return out
    out = np.stack([r["out"] for r in results], axis=0)
    return out


def kernel(cloud: np.ndarray) -> np.ndarray:
    return run(cloud)
